# revision 21
# baseline (speedup 1.0000x reference)
"""Trainium2 Bass kernel for nn_Encoder (2-layer GIN + BN + projection head).

Strategy (node/data parallel across 8 NeuronCores):
  - Nodes are block-partitioned across 8 cores (6250 each, global order).
  - Node-feature tables live in DRAM as bf16 [50000, 128]; int16 gather
    indices require splitting the table into two 25000-row banks (AP views).
  - Per core, in-edges (plus one self-edge per node, implementing GIN's
    z + aggregate) are sorted by destination and grouped by 128-destination
    segments ("groups"); each group's edges are padded to 128-edge tiles.
  - Edge features are fetched with non-transpose dma_gather (edge-major
    [128 edges x 128 feats] tiles).  Segment-sum uses the TensorEngine with
    swapped operands: psum[f, seg] += sum_e gather[e, f] * M[e, seg], where
    M is the one-hot destination matrix generated per tile by a DVE
    tensor_scalar is_equal against a constant iota row (4x mode).  The
    aggregation therefore lands feature-major, feeding the MLmatmuls with
    no transposes.
  - MLPs/BN/proj run feature-major: matmuls with pre-transposed weights,
    per-partition bias/scale via ScalarE activation; PReLU = max(y, a*y).
  - Halo exchange: one AllGather of the per-core z0 shard (bf16) between
    the two GIN layers builds the layer-1 gather table.
  - Outputs are PE-transposed back to node-major and DMA'd out.
  - SPMD: all shape-like parameters (tile counts per group/bank) are
    cross-core maxima; shorter cores pad with idx=0 / locdst=-1 edges that
    the is_equal mask zeroes out.
"""

import os
import numpy as np
import ml_dtypes

BF16 = ml_dtypes.bfloat16

DIM = 128
N_CORES = 8
BN_EPS = 1e-5
GRP = 512          # destination nodes per segment-sum group / MLP supertile
GRP_PER_ST = 1     # groups per MLP supertile


# ---------------------------------------------------------------------------
# Host-side preprocessing
# ---------------------------------------------------------------------------

def _prep(x, edge_index, weights, nb=N_CORES):
    x = np.asarray(x, dtype=np.float32)
    ei = np.asarray(edge_index)
    n = x.shape[0]
    assert n % (2 * nb) == 0
    per = n // nb
    split = n // 2

    src = ei[0].astype(np.int64)
    dst = ei[1].astype(np.int64)
    # append self edges (GIN: h = z_self + sum_neighbors)
    src = np.concatenate([src, np.arange(n)])
    dst = np.concatenate([dst, np.arange(n)])

    ngrp = (per + GRP - 1) // GRP          # groups per core
    # table position: [half][core][rank-within-half] so that an AllGather of
    # each shard half produces a contiguous table bank
    hp = per // 2                           # chunk-0 rows per core
    c0 = nb * hp                            # table rows in chunk 0
    def pos_of(v):
        k = v // per
        r = v - k * per
        half = (r >= hp).astype(np.int64)
        return half * c0 + k * np.where(half, per - hp, hp) + (r - half * hp)
    spos = pos_of(src)
    bank = (spos >= split).astype(np.int64)
    core = dst // per
    r = dst - core * per                    # rank within core
    grp = r // GRP                          # group within core
    seg = r % GRP                           # segment within group
    gkey = (core * ngrp + grp) * 2 + bank
    order = np.lexsort((seg, gkey))
    s_loc = (spos - bank * split)[order]    # bank-local source row
    s_seg = seg[order]
    counts = np.bincount(gkey, minlength=nb * ngrp * 2).reshape(nb, ngrp, 2)
    starts = np.concatenate([[0], np.cumsum(counts.reshape(-1))])[:-1].reshape(
        nb, ngrp, 2)

    # shared tile plan: per (group, bank) 128-edge tile count = max over cores
    tiles = np.maximum(np.ceil(counts / 128).astype(np.int64).max(axis=0), 1)
    t_a = tiles[:, 0]                       # [ngrp]
    t_b = tiles[:, 1]
    tot_t = int(t_a.sum() + t_b.sum())

    # per-(group,bank,tile) seg window, uniform across cores:
    # s0 = min over cores of the tile's min seg, span covers all cores
    s0_all = np.full((ngrp, 2, int(max(t_a.max(), t_b.max()))), GRP, np.int64)
    s1_all = np.full_like(s0_all, -1)
    for k in range(nb):
        for g in range(ngrp):
            for b in (0, 1):
                c = int(counts[k, g, b])
                st0 = int(starts[k, g, b])
                segs = s_seg[st0:st0 + c]
                tcnt = int((t_a if b == 0 else t_b)[g])
                for t in range(tcnt):
                    e0, e1 = t * 128, min((t + 1) * 128, c)
                    if e0 >= e1:
                        continue
                    s0_all[g, b, t] = min(s0_all[g, b, t], int(segs[e0]))
                    s1_all[g, b, t] = max(s1_all[g, b, t], int(segs[e1 - 1]))

    # supertile plan (shared with the builder)
    sts = []
    g = 0
    while g < ngrp:
        sts.append((g, min(GRP_PER_ST, ngrp - g)))
        g += GRP_PER_ST
    spans = np.where(s1_all >= 0, s1_all - s0_all + 1, 1)
    # batch M width per (supertile, bank) = max tile span within it
    st_sa = [max(int(spans[g, 0, t]) for g in range(g0, g0 + gs)
                 for t in range(int(t_a[g]))) for g0, gs in sts]
    st_sb = [max(int(spans[g, 1, t]) for g in range(g0, g0 + gs)
                 for t in range(int(t_b[g]))) for g0, gs in sts]
    st_of_g = {}
    for sidx, (g0, gs) in enumerate(sts):
        for g in range(g0, g0 + gs):
            st_of_g[g] = sidx
    # clamp each tile's base so [s0, s0 + batch_width) stays inside [0, GRP)
    tile_s0 = {}
    for g in range(ngrp):
        for b in (0, 1):
            width = (st_sa if b == 0 else st_sb)[st_of_g[g]]
            tcnt = int((t_a if b == 0 else t_b)[g])
            for t in range(tcnt):
                s0 = int(min(s0_all[g, b, t], GRP - width))
                tile_s0[(g, b, t)] = max(s0, 0)

    # per-core linear edge streams (bank A and bank B), group-major
    idx_all = {}
    ldst_all = {}
    for k in range(nb):
        lin_i = np.zeros(tot_t * 128, np.int64)
        lin_d = np.full(tot_t * 128, -1.0, np.float32)
        off = 0
        for b, tcnt in ((0, t_a), (1, t_b)):
            for g in range(ngrp):
                c = int(counts[k, g, b])
                st0 = int(starts[k, g, b])
                tn = int(tcnt[g])
                # seg relative to the tile's clamped base
                rel = np.full(tn * 128, -1.0, np.float32)
                for t in range(tn):
                    e0, e1 = t * 128, min((t + 1) * 128, c)
                    if e0 < e1:
                        rel[e0:e1] = (s_seg[st0 + e0:st0 + e1]
                                      - tile_s0[(g, b, t)])
                lin_i[off:off + c] = s_loc[st0:st0 + c]
                lin_d[off:off + tn * 128] = rel
                off += tn * 128
        assert lin_i.max() < 32768
        wi = lin_i.reshape(-1, 16).T.astype(np.int16)       # [16, tot_t*8]
        idx_all[k] = np.tile(wi, (8, 1))                    # [128, tot_t*8]
        ldst_all[k] = np.ascontiguousarray(
            lin_d.reshape(-1, 128).T.astype(np.float16))    # [128, tot_t]

    xtab = np.empty((n, DIM), BF16)
    xtab[pos_of(np.arange(n))] = x.astype(BF16)

    w = {k_: np.asarray(v, np.float32) for k_, v in weights.items()}
    bn_sc = w["bn_gamma"] / np.sqrt(w["bn_var"] + BN_EPS)
    bn_sh = w["bn_beta"] - w["bn_mean"] * bn_sc
    pp_a = w["pbn_gamma"] / np.sqrt(w["pbn_var"] + BN_EPS)
    pp_b = (w["proj_b"] - w["pbn_mean"]) * pp_a + w["pbn_beta"]

    col = lambda v: np.ascontiguousarray(v.reshape(DIM, 1), dtype=np.float32)
    wt = lambda m: np.ascontiguousarray(m.T, dtype=np.float32).astype(BF16)
    iota = np.tile(np.arange(GRP, dtype=np.float32), (DIM, 1)).astype(np.float16)

    shared = {
        "xtab": xtab, "iota": iota,
        "w1t0": wt(w["l0_w1"]), "w2t0": wt(w["l0_w2"]),
        "w1t1": wt(w["l1_w1"]), "w2t1": wt(w["l1_w2"]),
        "pwt": wt(w["proj_w"]),
        "b10": col(w["l0_b1"]), "b20": col(w["l0_b2"]),
        "b11": col(w["l1_b1"]), "b21": col(w["l1_b2"]),
        "bnsc": col(bn_sc), "bnsh": col(bn_sh),
        "ppa": col(pp_a), "ppb": col(pp_b),
    }
    in_maps = [dict(shared, idx=idx_all[k], ldst=ldst_all[k])
               for k in range(nb)]

    cfg = {
        "nb": nb, "n": n, "per": per, "split": split, "ngrp": ngrp, "hp": hp,
        "t_a": [int(v) for v in t_a], "t_b": [int(v) for v in t_b],
        "tot_t": tot_t,
        "tile_s0": {f"{g}_{b}_{t}": v for (g, b, t), v in tile_s0.items()},
        "st_sa": st_sa, "st_sb": st_sb,
        "alpha": float(np.asarray(w["prelu_a"]).reshape(-1)[0]),
    }
    return cfg, in_maps


# ---------------------------------------------------------------------------
# Device graph
# ---------------------------------------------------------------------------

def _build(cfg):
    import concourse.bass as bass
    import concourse.mybir as mybir
    import concourse.bacc as bacc
    import concourse.tile as tile

    dt = mybir.dt
    AF = mybir.ActivationFunctionType
    ALU = mybir.AluOpType
    nb, n, per, split = cfg["nb"], cfg["n"], cfg["per"], cfg["split"]
    ngrp, t_a, t_b, tot_t = cfg["ngrp"], cfg["t_a"], cfg["t_b"], cfg["tot_t"]
    alpha = cfg["alpha"]
    tile_s0 = {tuple(int(x) for x in k.split("_")): v
               for k, v in cfg["tile_s0"].items()}
    st_sa, st_sb = cfg["st_sa"], cfg["st_sb"]

    nc = bacc.Bacc("TRN2", target_bir_lowering=False, debug=False,
                   enable_asserts=False, num_devices=nb,
                   num_swdge_queues=4)

    xtab = nc.dram_tensor("xtab", [n, DIM], dt.bfloat16, kind="ExternalInput")
    iota_in = nc.dram_tensor("iota", [DIM, GRP], dt.float16, kind="ExternalInput")
    idx_in = nc.dram_tensor("idx", [128, tot_t * 8], dt.int16, kind="ExternalInput")
    ldst_in = nc.dram_tensor("ldst", [128, tot_t], dt.float16, kind="ExternalInput")
    wts = {nm: nc.dram_tensor(nm, [DIM, DIM], dt.bfloat16, kind="ExternalInput")
           for nm in ("w1t0", "w2t0", "w1t1", "w2t1", "pwt")}
    cols = {nm: nc.dram_tensor(nm, [DIM, 1], dt.float32, kind="ExternalInput")
            for nm in ("b10", "b20", "b11", "b21", "bnsc", "bnsh", "ppa", "ppb")}

    zout = nc.dram_tensor("zout", [per, DIM], dt.float32, kind="ExternalOutput")
    pout = nc.dram_tensor("pout", [per, DIM], dt.float32, kind="ExternalOutput")
    zshard = nc.dram_tensor("zshard", [per, DIM], dt.bfloat16)
    ztab = nc.dram_tensor("ztab", [n, DIM], dt.bfloat16, addr_space="Shared")

    # supertile plan: [(g0, ngroups)] covering all ngrp groups
    sts = []
    g = 0
    while g < ngrp:
        sts.append((g, min(GRP_PER_ST, ngrp - g)))
        g += GRP_PER_ST
    # per-supertile bank tile counts and SBUF buffer sizing
    st_ta = [sum(t_a[g0:g0 + gs]) for g0, gs in sts]
    st_tb = [sum(t_b[g0:g0 + gs]) for g0, gs in sts]
    max_ta, max_tb = max(st_ta), max(st_tb)
    max_sa, max_sb = max(st_sa), max(st_sb)

    with tile.TileContext(nc) as tc:
        with (
            tc.tile_pool(name="const", bufs=1) as const,
            tc.tile_pool(name="slota", bufs=4) as pool_a,
            tc.tile_pool(name="slotb", bufs=4) as pool_b,
            tc.tile_pool(name="mpool", bufs=4) as mpool,
            tc.tile_pool(name="act", bufs=3) as act_p,
            tc.tile_pool(name="stage", bufs=3) as stage_p,
            tc.tile_pool(name="psseg", bufs=3, space="PSUM") as ps_seg,
            tc.tile_pool(name="psmm", bufs=2, space="PSUM") as ps_mm,
            tc.tile_pool(name="pstr", bufs=3, space="PSUM") as ps_tr,
        ):
            idx_sb = const.tile([128, tot_t * 8], dt.int16, tag="idx")
            nc.sync.dma_start(out=idx_sb[:], in_=idx_in[:])
            iota_sb = const.tile([DIM, GRP], dt.float16, tag="iota")
            nc.sync.dma_start(out=iota_sb[:], in_=iota_in[:])
            wt_t = {}
            for nm, h in wts.items():
                t = const.tile([DIM, DIM], dt.bfloat16, tag=nm)
                nc.sync.dma_start(out=t[:], in_=h[:])
                wt_t[nm] = t
            col_t = {}
            for nm, h in cols.items():
                t = const.tile([DIM, 1], dt.float32, tag=nm)
                nc.sync.dma_start(out=t[:], in_=h[:])
                col_t[nm] = t
            ldst_sb = const.tile([128, tot_t], dt.float16, tag="ldst")
            nc.sync.dma_start(out=ldst_sb[:], in_=ldst_in[:])

            # identity for PE transposes
            from concourse.masks import make_identity
            ident = const.tile([128, 128], dt.bfloat16, tag="ident")
            make_identity(nc, ident[:])

            def store_rows(src_bf16, base_row, rows, out_h, out_dt):
                """PE-transpose a [128, 128] feature-major slice and DMA the
                first `rows` node-major rows to out_h[base_row:...]."""
                pt = ps_tr.tile([128, 128], dt.bfloat16, tag="tr")
                nc.tensor.transpose(pt[:], src_bf16, ident[:])
                st = stage_p.tile([128, 128], out_dt, tag="ost")
                nc.scalar.copy(st[:], pt[:])
                nc.sync.dma_start(out=out_h[base_row:base_row + rows, :],
                                  in_=st[0:rows, :])

            for layer in (0, 1):
                tab = xtab if layer == 0 else ztab
                tab_a = tab[0:split, :]
                tab_b = tab[split:n, :]
                w1 = wt_t["w1t0" if layer == 0 else "w1t1"]
                w2 = wt_t["w2t0" if layer == 0 else "w2t1"]
                b1 = col_t["b10" if layer == 0 else "b11"]
                b2 = col_t["b20" if layer == 0 else "b21"]

                # edge/tile offsets: bank A stream first, then bank B
                offe_a = 0
                offt_a = 0
                offe_b = sum(t_a) * 128
                offt_b = sum(t_a)
                for si, (g0, gs) in enumerate(sts):
                    ta, tbk = st_ta[si], st_tb[si]
                    nsa, nsb = ta * 128, tbk * 128
                    sa = pool_a.tile([128, max_ta, DIM], dt.bfloat16, tag="sa")
                    sb = pool_b.tile([128, max_tb, DIM], dt.bfloat16, tag="sb")
                    qp_a, qp_b = ((0, 1), (2, 3)) if si % 2 == 0 else \
                                 ((2, 3), (0, 1))
                    for (tcnt, slot, tabx, offe, qpair) in (
                        (ta, sa, tab_a, offe_a, qp_a),
                        (tbk, sb, tab_b, offe_b, qp_b),
                    ):
                        h1 = (tcnt + 1) // 2
                        for (lo, hi, q) in ((0, h1, qpair[0]),
                                            (h1, tcnt, qpair[1])):
                            if hi <= lo:
                                continue
                            ns = (hi - lo) * 128
                            e0 = offe + lo * 128
                            nc.gpsimd.dma_gather(
                                out_ap=slot[:, lo:hi, :], in_ap=tabx,
                                idxs_ap=idx_sb[:, e0 // 16:(e0 + ns) // 16],
                                num_idxs=ns, num_idxs_reg=ns, elem_size=DIM,
                                transpose=False, single_packet=False,
                                queue_num=q)

                    nst = min(GRP, ((per - g0 * GRP + 127) // 128) * 128)
                    # batched narrow one-hot M: M[e, t, s] = (rel[e,t] == s)
                    S_a, S_b = st_sa[si], st_sb[si]
                    ma = mpool.tile([128, max_ta, max_sa], dt.bfloat16,
                                    tag="ma")
                    nc.vector.tensor_tensor(
                        out=ma[:, 0:ta, 0:S_a],
                        in0=ldst_sb[:, offt_a:offt_a + ta, None]
                            .to_broadcast([128, ta, S_a]),
                        in1=iota_sb[:, None, 0:S_a].to_broadcast(
                            [128, ta, S_a]),
                        op=ALU.is_equal)
                    mb = mpool.tile([128, max_tb, max_sb], dt.bfloat16,
                                    tag="mb")
                    nc.vector.tensor_tensor(
                        out=mb[:, 0:tbk, 0:S_b],
                        in0=ldst_sb[:, offt_b:offt_b + tbk, None]
                            .to_broadcast([128, tbk, S_b]),
                        in1=iota_sb[:, None, 0:S_b].to_broadcast(
                            [128, tbk, S_b]),
                        op=ALU.is_equal)
                    h_sb = act_p.tile([128, GRP], dt.bfloat16, tag="h")
                    g = g0
                    gta, gtb = t_a[g], t_b[g]
                    ps = ps_seg.tile([128, GRP], dt.float32, tag="seg")
                    nc.scalar.memzero(ps[:])
                    nmm = gta + gtb
                    imm = 0
                    for b, gt, slot, mbuf, lcur, S in (
                        (0, gta, sa, ma, 0, S_a),
                        (1, gtb, sb, mb, 0, S_b),
                    ):
                        for t in range(gt):
                            s0 = tile_s0[(g, b, t)]
                            nc.tensor.matmul(
                                ps[:, s0:s0 + S],
                                lhsT=slot[:, lcur + t, :],
                                rhs=mbuf[:, lcur + t, 0:S],
                                start=False, stop=(imm == nmm - 1),
                                skip_group_check=True)
                            imm += 1
                    nc.scalar.copy(h_sb[:, 0:nst], ps[:, 0:nst])
                    offe_a += nsa
                    offt_a += ta
                    offe_b += nsb
                    offt_b += tbk

                    # MLP on the supertile (feature-major)
                    h_ap = h_sb[:, 0:nst]
                    ps1 = ps_mm.tile([128, nst], dt.float32, tag="mm")
                    nc.tensor.matmul(ps1[:], lhsT=w1[:], rhs=h_ap,
                                     start=True, stop=True)
                    h1 = act_p.tile([128, GRP], dt.bfloat16, tag="h1")
                    nc.scalar.activation(h1[:, 0:nst], ps1[:], AF.Relu,
                                         bias=b1[:])
                    ps2 = ps_mm.tile([128, nst], dt.float32, tag="mm")
                    nc.tensor.matmul(ps2[:], lhsT=w2[:], rhs=h1[:, 0:nst],
                                     start=True, stop=True)
                    z = act_p.tile([128, GRP], dt.bfloat16, tag="z")
                    nc.scalar.activation(z[:, 0:nst], ps2[:], AF.Relu,
                                         bias=b2[:])

                    base = g0 * GRP
                    if layer == 0:
                        for c in range(GRP // 128):
                            r0 = base + c * 128
                            rows = min(128, per - r0)
                            if rows > 0:
                                store_rows(z[:, c * 128:(c + 1) * 128], r0,
                                           rows, zshard, dt.bfloat16)
                        hp = cfg["hp"]
                        c0 = nb * hp
                        if base < hp <= base + GRP:
                            # first shard chunk complete: all-gather it
                            nc.gpsimd.collective_compute(
                                "AllGather", mybir.AluOpType.bypass,
                                replica_groups=[list(range(nb))],
                                ins=[zshard[0:hp, :]], outs=[ztab[0:c0, :]])
                        if si == len(sts) - 1 and hp < per:
                            nc.gpsimd.collective_compute(
                                "AllGather", mybir.AluOpType.bypass,
                                replica_groups=[list(range(nb))],
                                ins=[zshard[hp:per, :]],
                                outs=[ztab[c0:n, :]])
                    else:
                        zbn = act_p.tile([128, GRP], dt.bfloat16, tag="zbn")
                        nc.scalar.activation(zbn[:, 0:nst], z[:, 0:nst],
                                             AF.Identity,
                                             bias=col_t["bnsh"][:],
                                             scale=col_t["bnsc"][:])
                        ps3 = ps_mm.tile([128, nst], dt.float32, tag="mm")
                        nc.tensor.matmul(ps3[:], lhsT=wt_t["pwt"][:],
                                         rhs=zbn[:, 0:nst], start=True,
                                         stop=True)
                        y = act_p.tile([128, GRP], dt.bfloat16, tag="y")
                        nc.scalar.activation(y[:, 0:nst], ps3[:], AF.Identity,
                                             bias=col_t["ppb"][:],
                                             scale=col_t["ppa"][:])
                        ya = act_p.tile([128, GRP], dt.bfloat16, tag="ya")
                        nc.vector.tensor_scalar(out=ya[:, 0:nst],
                                                in0=y[:, 0:nst],
                                                scalar1=alpha, scalar2=None,
                                                op0=ALU.mult)
                        pp = act_p.tile([128, GRP], dt.bfloat16, tag="pp")
                        nc.vector.tensor_tensor(out=pp[:, 0:nst],
                                                in0=y[:, 0:nst],
                                                in1=ya[:, 0:nst], op=ALU.max)
                        for c in range(GRP // 128):
                            r0 = base + c * 128
                            rows = min(128, per - r0)
                            if rows > 0:
                                store_rows(zbn[:, c * 128:(c + 1) * 128], r0,
                                           rows, zout, dt.float32)
                                store_rows(pp[:, c * 128:(c + 1) * 128], r0,
                                           rows, pout, dt.float32)



    nc.compile()
    return nc


# ---------------------------------------------------------------------------
# Entry point
# ---------------------------------------------------------------------------

_WEIGHT_KEYS = (
    "l0_w1", "l0_b1", "l0_w2", "l0_b2", "l1_w1", "l1_b1", "l1_w2", "l1_b2",
    "bn_gamma", "bn_beta", "bn_mean", "bn_var", "proj_w", "proj_b",
    "pbn_gamma", "pbn_beta", "pbn_mean", "pbn_var", "prelu_a",
)

last_exec_ns = None


def _install_ntff_shim():
    """Provide the antenv.axon_hooks module bass_utils expects for
    trace=True under axon, backed by trn_agent_boot's ctypes hook."""
    import sys
    import types
    if "antenv.axon_hooks" in sys.modules:
        return
    try:
        from trn_agent_boot.trn_boot import _ntff_profile_via_ctypes
        hook = _ntff_profile_via_ctypes("/opt/axon/libaxon_pjrt.so")
    except Exception:
        hook = None
    mod = types.ModuleType("antenv.axon_hooks")
    mod._hook = hook
    mod.get_axon_ntff_profile_hook = lambda: mod._hook
    mod.set_axon_ntff_profile_hook = lambda h: setattr(mod, "_hook", h)
    sys.modules["antenv.axon_hooks"] = mod


def kernel(x, edge_index, **weights):
    global last_exec_ns
    from concourse.bass_utils import run_bass_kernel_spmd

    weights = {k: np.asarray(weights[k]) for k in _WEIGHT_KEYS}
    cfg, in_maps = _prep(np.asarray(x), np.asarray(edge_index), weights)
    nc = _build(cfg)

    trace = bool(int(os.environ.get("GNN_PROFILE", "0")))
    if trace:
        _install_ntff_shim()
    res = run_bass_kernel_spmd(nc, in_maps, list(range(cfg["nb"])), trace=trace)
    last_exec_ns = res.exec_time_ns

    z = np.concatenate([res.results[k]["zout"] for k in range(cfg["nb"])])
    p = np.concatenate([res.results[k]["pout"] for k in range(cfg["nb"])])
    return z, p
